# revision 34
# baseline (speedup 1.0000x reference)
"""Trainium2 Bass kernel for nn_AttentionBase (8-head attention w/ T5-style
relative-position bias + output projection), sharded head-parallel over 8
NeuronCores.

Host side (per core c, head h=c): q,k are sliced to head h, transposed to
[2, 64, n] and cast bf16; v is laid out as [2, 128, Kt*(65)] bf16 with a
ones-column (softmax-denominator trick); the bias Toeplitz
expP[r, c] = exp(SCALE * table[bucket(key-query)]) is fully materialized as
[128, 2n-128] fp16 on host.

Device program per core (single head, both batches):
  attention: for each step s=(qb, kt) over query blocks (w=1024) x key tiles:
      S^T[128, w] = kT_tile^T.T @ qT          (PE, bf16 in, fp32 PSUM)
      araw = exp(SCALE * S^T)                 (ACT, PSUM->SBUF bf16)
      at   = araw * expP_slice                (DVE, 2x 16-bit mode)
      outT[65, w] += V'[kt]^T.T @ at          (PE, lagged LAG steps behind QK
                                               so ACT/DVE latency never stalls
                                               the in-order PE queue)
  per query block: copy out to SBUF, bounce denominator row through DRAM to
  transpose it to [128, 8], reciprocal on DVE, then per 128-query tile:
  project with W_h (PE) and scale by 1/den during the PSUM->SBUF copy (DVE
  tensor_scalar), DMA to DRAM.  Projections are interleaved 1-per-2-steps
  into the subsequent attention steps so the ACT engine (the bottleneck:
  n^2 exps at 128 lanes/cycle) never idles.
Host: out = sum_c partial_c + b_out.
"""

import math
import sys

sys.path.insert(0, "/opt/trn_rl_repo")

import numpy as np
import ml_dtypes

import concourse.bass as bass
import concourse.bacc as bacc_mod
import concourse.mybir as mybir
import concourse.tile as tile

NUM_HEADS = 8
HEAD_DIM = 64
MID = 512
OUT_F = 512
NUM_BUCKETS = 32
MAX_DISTANCE = 128
SCALE = HEAD_DIM ** -0.5
N_CORES = 8

F32 = mybir.dt.float32
BF16 = mybir.dt.bfloat16
F32R = mybir.dt.float32r
F16 = mybir.dt.float16
AF = mybir.ActivationFunctionType


def _bucket_np(rel):
    """Exact numpy port of reference._relative_position_bucket with
    num_buckets=64, max_distance=128 (as the module calls it)."""
    num_buckets = (2 * NUM_BUCKETS) // 2  # 32
    ret = (rel >= 0).astype(np.int32) * num_buckets
    n = np.abs(rel)
    max_exact = max(1, num_buckets // 2)  # 16
    denom = (
        math.log(MAX_DISTANCE / max_exact) if MAX_DISTANCE > max_exact else 1.0
    )
    n_float = np.maximum(n.astype(np.float32), 1.0)
    val_if_large = (
        max_exact + np.log(n_float / max_exact) / denom * (num_buckets - max_exact)
    ).astype(np.int32)
    val_if_large = np.minimum(val_if_large, num_buckets - 1)
    return ret + np.where(n < max_exact, n, val_if_large)


def _expP_np(table_col, n):
    """[128, 2n-128] fp16 Toeplitz: expP[r, c] = exp(SCALE * f(r - c + n - 128))
    where f(d) = table[bucket(d)].  Slice [:, c0:c0+w] with
    c0 = (n-128) - 128*kt + w*qb gives exp(SCALE * bias[key, query])."""
    x = np.arange(2 * n, dtype=np.int64)
    d = (n - 1) - x
    e = np.exp(SCALE * table_col[_bucket_np(d)]).astype(np.float32)  # Frev
    Wp = 2 * n - 128
    idx = (127 - np.arange(128, dtype=np.int64))[:, None] + np.arange(
        Wp, dtype=np.int64
    )[None, :]
    return e[idx].astype(np.float16)


def build_nc(n=4096, w=1024):
    assert n % 128 == 0 and n % w == 0
    Kt = n // 128           # key tiles
    n_qb = n // w           # query blocks per batch
    nsteps = n_qb * Kt      # attention steps per batch
    qb_t = w // 128         # query tiles per query block
    Wp = 2 * n - 128
    LAG = 2                 # AV trails QK by this many steps on the PE queue
    VW = HEAD_DIM + 1       # V' valid width (ones column at 64)
    VP = 128                # V' padded width: 128 weights enables FWL

    nc = bacc_mod.Bacc()
    qT_h = nc.declare_dram_parameter("qT_h", [2, HEAD_DIM, n], BF16, isOutput=False)
    kT_h = nc.declare_dram_parameter("kT_h", [2, HEAD_DIM, n], BF16, isOutput=False)
    vp_h = nc.declare_dram_parameter("vp_h", [2, 128, Kt * VP], BF16, isOutput=False)
    expP_h = nc.declare_dram_parameter("expP_h", [128, Wp], F16, isOutput=False)
    W_h = nc.declare_dram_parameter("W_h", [HEAD_DIM, OUT_F], BF16, isOutput=False)
    out_p = nc.declare_dram_parameter(
        "out_partial", [2, n, OUT_F], BF16, isOutput=True
    )
    den_scr = nc.dram_tensor("den_scr", (2, n), F32)

    with tile.TileContext(nc) as tc:
        with (
            tc.tile_pool(name="const", bufs=1) as constp,
            tc.tile_pool(name="qkT", bufs=2) as qkTp,
            tc.tile_pool(name="vpp", bufs=2) as vpp,
            tc.tile_pool(name="o65p", bufs=2) as o65p,
            tc.tile_pool(name="stage", bufs=6) as stage,
            tc.tile_pool(name="atp", bufs=LAG + 4) as atp,
            tc.tile_pool(name="osbp", bufs=6) as osbp,
            tc.tile_pool(name="recp", bufs=3) as recp,
            tc.tile_pool(name="pst", bufs=2, space="PSUM") as pst,
            tc.tile_pool(name="pav", bufs=2, space="PSUM") as pav,
        ):
            # PE warm-up burst: ~12 dependency-free matmuls on a memset tile
            # flip the HAM clock gate (K=4/8 -> 8/8) before real work arrives;
            # without this the PE runs at 1.2 GHz for ~100us into the kernel
            warm = constp.tile([HEAD_DIM, 512], BF16, tag="warm")
            nc.gpsimd.memset(warm, 0.0)
            wpsum = pst.tile([128, 512], F32, tag="st", name="wpsum")
            for _ in range(4):
                nc.tensor.matmul(
                    wpsum, warm[:, 0:128], warm[:, :], start=True, stop=True
                )

            # the 2MB expP streams on the sync queue (alongside the later
            # output stores) so the first q/k chunks on the gpsimd queue
            # aren't delayed behind it; expP is only needed by the first mul,
            # which the araw buffering absorbs
            expP = constp.tile([128, Wp], F16, tag="expP")
            nc.sync.dma_start(expP, expP_h[:, :])
            Wt = constp.tile([HEAD_DIM, OUT_F], BF16, tag="W")
            nc.sync.dma_start(Wt, W_h[:, :])

            proj_jobs = []  # deferred per-query-tile projection closures

            def make_pj(b, qt, o65, rec, rj):
                def issue():
                    pj = pav.tile([128, OUT_F], F32, tag="avpj")
                    nc.tensor.matmul(
                        pj,
                        o65[0:HEAD_DIM, 128 * qt : 128 * (qt + 1)],
                        Wt,
                        start=True,
                        stop=True,
                    )
                    osb = osbp.tile([128, OUT_F], BF16, tag="osb")
                    nc.vector.tensor_scalar_mul(osb, pj, rec[:, rj : rj + 1])
                    nc.sync.dma_start(out_p[b, 128 * qt : 128 * (qt + 1), :], osb)

                return issue

            for b in range(2):
                # chunked loads so the first QK only waits on the first chunk
                qT = qkTp.tile([HEAD_DIM, n], BF16, tag="qT")
                kT = qkTp.tile([HEAD_DIM, n], BF16, tag="kT")
                for ch in range(4):
                    cs = slice(ch * n // 4, (ch + 1) * n // 4)
                    nc.gpsimd.dma_start(kT[:, cs], kT_h[b][:, cs])
                    nc.gpsimd.dma_start(qT[:, cs], qT_h[b][:, cs])
                vp = vpp.tile([128, Kt * VP], BF16, tag="vp")
                nc.gpsimd.dma_start(vp, vp_h[b])

                o65 = o65p.tile([HEAD_DIM, n], BF16, tag="o65")
                ats = {}
                avs = {}

                def issue_qk(s, qT=qT, kT=kT, ats=ats):
                    qb, kt = divmod(s, Kt)
                    st = pst.tile([128, w], F32, tag="st")
                    for h in range(w // 512):
                        nc.tensor.matmul(
                            st[:, 512 * h : 512 * (h + 1)],
                            kT[:, 128 * kt : 128 * (kt + 1)],
                            qT[:, w * qb + 512 * h : w * qb + 512 * (h + 1)],
                            start=True,
                            stop=True,
                        )
                    araw = stage.tile([128, w], BF16, tag="araw")
                    nc.scalar.activation(araw, st, AF.Exp, scale=SCALE)
                    at = atp.tile([128, w], BF16, tag="at")
                    c0 = (n - 128) - 128 * kt + w * qb
                    nc.vector.tensor_mul(at, araw, expP[:, c0 : c0 + w])
                    ats[s] = at

                def issue_av(s, b=b, vp=vp, o65=o65, ats=ats, avs=avs):
                    qb, kt = divmod(s, Kt)
                    if kt == 0:
                        avs[qb] = pav.tile([128, w], F32, tag="avpj", name="av")
                    av = avs[qb]
                    at = ats.pop(s)
                    for h in range(w // 512):
                        nc.tensor.matmul(
                            av[:, 512 * h : 512 * (h + 1)],
                            vp[:, VP * kt : VP * (kt + 1)],
                            at[:, 512 * h : 512 * (h + 1)],
                            start=(kt == 0),
                            stop=(kt == Kt - 1),
                        )
                    if kt == Kt - 1:
                        # denominator row first (exact f32) so its DRAM
                        # bounce chain starts before the big output copy
                        densb = recp.tile([1, w], F32, tag="densb")
                        nc.vector.tensor_copy(
                            densb, av[HEAD_DIM : HEAD_DIM + 1, :]
                        )
                        nc.gpsimd.dma_start(
                            den_scr[b : b + 1, w * qb : w * (qb + 1)], densb
                        )
                        nc.vector.tensor_copy(
                            o65[:, w * qb : w * (qb + 1)], av[0:HEAD_DIM, :]
                        )
                        del avs[qb]
                        denT = recp.tile([128, qb_t], F32, tag="denT")
                        bsrc = bass.AP(
                            tensor=den_scr,
                            offset=b * n + w * qb,
                            ap=[[1, 128], [128, qb_t]],
                        )
                        nc.gpsimd.dma_start(denT, bsrc)
                        rec = recp.tile([128, qb_t], F32, tag="rec")
                        nc.vector.reciprocal(rec, denT)
                        for rj in range(qb_t):
                            proj_jobs.append(
                                make_pj(b, qb_t * qb + rj, o65, rec, rj)
                            )

                for s in range(nsteps + LAG):
                    if s < nsteps:
                        issue_qk(s)
                    if s >= LAG:
                        issue_av(s - LAG)
                    # drain one projection every 4 steps: the extra PE matmul
                    # per 4 steps stays under the ACT exp pace.  The last
                    # query block's jobs intentionally spill into the next
                    # batch's loop — draining them here would stall the PE on
                    # the reciprocal latency and MID-rethrottle the clock gate
                    if proj_jobs and s % 4 == 1:
                        proj_jobs.pop(0)()

            while proj_jobs:
                proj_jobs.pop(0)()

    nc.compile()
    return nc


def make_in_maps(q, k, v, rel_bias_table, W_out, n):
    """Shard + pre-layout full inputs per core (core c <-> head c)."""
    Kt = n // 128
    in_maps = []
    for c in range(N_CORES):
        sl = slice(HEAD_DIM * c, HEAD_DIM * (c + 1))
        qT = np.ascontiguousarray(np.transpose(q[:, :, sl], (0, 2, 1))).astype(
            ml_dtypes.bfloat16
        )
        kT = np.ascontiguousarray(np.transpose(k[:, :, sl], (0, 2, 1))).astype(
            ml_dtypes.bfloat16
        )
        vr = v[:, :, sl].reshape(2, Kt, 128, HEAD_DIM)
        vp = np.zeros((2, 128, Kt, 128), dtype=ml_dtypes.bfloat16)
        vp[:, :, :, :HEAD_DIM] = np.transpose(vr, (0, 2, 1, 3)).astype(
            ml_dtypes.bfloat16
        )
        vp[:, :, :, HEAD_DIM] = 1.0
        in_maps.append(
            {
                "qT_h": qT,
                "kT_h": kT,
                "vp_h": np.ascontiguousarray(vp.reshape(2, 128, Kt * 128)),
                "expP_h": _expP_np(rel_bias_table[:, c].astype(np.float64), n),
                "W_h": np.ascontiguousarray(W_out[sl, :]).astype(
                    ml_dtypes.bfloat16
                ),
            }
        )
    return in_maps


_NC_CACHE = {}


def _get_nc(n, w):
    key = (n, w)
    if key not in _NC_CACHE:
        _NC_CACHE[key] = build_nc(n=n, w=w)
    return _NC_CACHE[key]


def kernel(q, k, v, rel_bias_table, W_out, b_out):
    from concourse.bass_utils import run_bass_kernel_spmd

    q = np.asarray(q, dtype=np.float32)
    k = np.asarray(k, dtype=np.float32)
    v = np.asarray(v, dtype=np.float32)
    rel_bias_table = np.asarray(rel_bias_table, dtype=np.float32)
    W_out = np.asarray(W_out, dtype=np.float32)
    b_out = np.asarray(b_out, dtype=np.float32)

    n = q.shape[1]
    w = min(1024, n)
    nc = _get_nc(n, w)
    in_maps = make_in_maps(q, k, v, rel_bias_table, W_out, n)
    res = run_bass_kernel_spmd(nc, in_maps, core_ids=list(range(N_CORES)))
    acc = np.zeros((2, n, OUT_F), dtype=np.float64)
    for r in res.results:
        acc += r["out_partial"].astype(np.float64)
    return (acc + b_out.astype(np.float64)).astype(np.float32)


# revision 39
# speedup vs baseline: 1.0625x; 1.0625x over previous
"""Trainium2 Bass kernel for nn_AttentionBase (8-head attention w/ T5-style
relative-position bias + output projection), sharded head-parallel over 8
NeuronCores.

Host side (per core c, head h=c): q,k are sliced to head h, transposed to
[2, 64, n] and cast bf16; v is laid out as [2, 128, Kt*(65)] bf16 with a
ones-column (softmax-denominator trick); the bias Toeplitz
expP[r, c] = exp(SCALE * table[bucket(key-query)]) is fully materialized as
[128, 2n-128] fp16 on host.

Device program per core (single head, both batches):
  attention: for each step s=(qb, kt) over query blocks (w=1024) x key tiles:
      S^T[128, w] = kT_tile^T.T @ qT          (PE, bf16 in, fp32 PSUM)
      araw = exp(SCALE * S^T)                 (ACT, PSUM->SBUF bf16)
      at   = araw * expP_slice                (DVE, 2x 16-bit mode)
      outT[65, w] += V'[kt]^T.T @ at          (PE, lagged LAG steps behind QK
                                               so ACT/DVE latency never stalls
                                               the in-order PE queue)
  per query block: copy out to SBUF, bounce denominator row through DRAM to
  transpose it to [128, 8], reciprocal on DVE, then per 128-query tile:
  project with W_h (PE) and scale by 1/den during the PSUM->SBUF copy (DVE
  tensor_scalar), DMA to DRAM.  Projections are interleaved 1-per-2-steps
  into the subsequent attention steps so the ACT engine (the bottleneck:
  n^2 exps at 128 lanes/cycle) never idles.
Host: out = sum_c partial_c + b_out.
"""

import math
import sys

sys.path.insert(0, "/opt/trn_rl_repo")

import numpy as np
import ml_dtypes

import concourse.bass as bass
import concourse.bacc as bacc_mod
import concourse.mybir as mybir
import concourse.tile as tile

NUM_HEADS = 8
HEAD_DIM = 64
MID = 512
OUT_F = 512
NUM_BUCKETS = 32
MAX_DISTANCE = 128
SCALE = HEAD_DIM ** -0.5
N_CORES = 8

F32 = mybir.dt.float32
BF16 = mybir.dt.bfloat16
F32R = mybir.dt.float32r
F16 = mybir.dt.float16
AF = mybir.ActivationFunctionType


def _bucket_np(rel):
    """Exact numpy port of reference._relative_position_bucket with
    num_buckets=64, max_distance=128 (as the module calls it)."""
    num_buckets = (2 * NUM_BUCKETS) // 2  # 32
    ret = (rel >= 0).astype(np.int32) * num_buckets
    n = np.abs(rel)
    max_exact = max(1, num_buckets // 2)  # 16
    denom = (
        math.log(MAX_DISTANCE / max_exact) if MAX_DISTANCE > max_exact else 1.0
    )
    n_float = np.maximum(n.astype(np.float32), 1.0)
    val_if_large = (
        max_exact + np.log(n_float / max_exact) / denom * (num_buckets - max_exact)
    ).astype(np.int32)
    val_if_large = np.minimum(val_if_large, num_buckets - 1)
    return ret + np.where(n < max_exact, n, val_if_large)


def _expP_np(table_col, n):
    """[128, 2n-128] fp16 Toeplitz: expP[r, c] = exp(SCALE * f(r - c + n - 128))
    where f(d) = table[bucket(d)].  Slice [:, c0:c0+w] with
    c0 = (n-128) - 128*kt + w*qb gives exp(SCALE * bias[key, query])."""
    x = np.arange(2 * n, dtype=np.int64)
    d = (n - 1) - x
    e = np.exp(SCALE * table_col[_bucket_np(d)]).astype(np.float32)  # Frev
    Wp = 2 * n - 128
    idx = (127 - np.arange(128, dtype=np.int64))[:, None] + np.arange(
        Wp, dtype=np.int64
    )[None, :]
    return e[idx].astype(np.float16)


def build_nc(n=4096, w=1024):
    assert n % 128 == 0 and n % w == 0
    Kt = n // 128           # key tiles
    n_qb = n // w           # query blocks per batch
    nsteps = n_qb * Kt      # attention steps per batch
    qb_t = w // 128         # query tiles per query block
    Wp = 2 * n - 128
    LAG = 3                 # AV trails QK by this many steps on the PE queue
    VW = HEAD_DIM + 1       # V' valid width (ones column at 64)
    VP = 128                # V' padded width: 128 weights enables FWL

    nc = bacc_mod.Bacc()
    qT_h = nc.declare_dram_parameter("qT_h", [2, HEAD_DIM, n], BF16, isOutput=False)
    kT_h = nc.declare_dram_parameter("kT_h", [2, HEAD_DIM, n], BF16, isOutput=False)
    vp_h = nc.declare_dram_parameter("vp_h", [2, 128, Kt * VP], BF16, isOutput=False)
    expP_h = nc.declare_dram_parameter("expP_h", [128, Wp], F16, isOutput=False)
    W_h = nc.declare_dram_parameter("W_h", [HEAD_DIM, OUT_F], BF16, isOutput=False)
    out_p = nc.declare_dram_parameter(
        "out_partial", [2, n, OUT_F], BF16, isOutput=True
    )
    den_scr = nc.dram_tensor("den_scr", (2, n), F32)

    with tile.TileContext(nc) as tc:
        with (
            tc.tile_pool(name="const", bufs=1) as constp,
            tc.tile_pool(name="qkT", bufs=2) as qkTp,
            tc.tile_pool(name="vpp", bufs=2) as vpp,
            tc.tile_pool(name="o65p", bufs=2) as o65p,
            tc.tile_pool(name="stage", bufs=6) as stage,
            tc.tile_pool(name="atp", bufs=LAG + 4) as atp,
            tc.tile_pool(name="osbp", bufs=6) as osbp,
            tc.tile_pool(name="recp", bufs=3) as recp,
            tc.tile_pool(name="pst", bufs=2, space="PSUM") as pst,
            tc.tile_pool(name="pav", bufs=2, space="PSUM") as pav,
        ):
            # PE warm-up burst: ~12 dependency-free matmuls on a memset tile
            # flip the HAM clock gate (K=4/8 -> 8/8) before real work arrives;
            # without this the PE runs at 1.2 GHz for ~100us into the kernel
            warm = constp.tile([HEAD_DIM, 512], BF16, tag="warm")
            nc.gpsimd.memset(warm, 0.0)
            wpsum = pst.tile([128, 512], F32, tag="st", name="wpsum")
            for _ in range(4):
                nc.tensor.matmul(
                    wpsum, warm[:, 0:128], warm[:, :], start=True, stop=True
                )

            # the 2MB expP streams on the sync queue (alongside the later
            # output stores) so the first q/k chunks on the gpsimd queue
            # aren't delayed behind it; expP is only needed by the first mul,
            # which the araw buffering absorbs
            expP = constp.tile([128, Wp], F16, tag="expP")
            nc.sync.dma_start(expP, expP_h[:, :])
            Wt = constp.tile([HEAD_DIM, OUT_F], BF16, tag="W")
            nc.sync.dma_start(Wt, W_h[:, :])

            proj_jobs = []  # deferred per-query-tile projection closures

            def make_pj(b, qt, o65, rec, rj):
                def issue():
                    pj = pav.tile([128, OUT_F], F32, tag="avpj")
                    nc.tensor.matmul(
                        pj,
                        o65[0:HEAD_DIM, 128 * qt : 128 * (qt + 1)],
                        Wt,
                        start=True,
                        stop=True,
                    )
                    osb = osbp.tile([128, OUT_F], BF16, tag="osb")
                    nc.vector.tensor_scalar_mul(osb, pj, rec[:, rj : rj + 1])
                    nc.sync.dma_start(out_p[b, 128 * qt : 128 * (qt + 1), :], osb)

                return issue

            for b in range(2):
                # chunked loads so the first QK only waits on the first chunk
                qT = qkTp.tile([HEAD_DIM, n], BF16, tag="qT")
                kT = qkTp.tile([HEAD_DIM, n], BF16, tag="kT")
                for ch in range(4):
                    cs = slice(ch * n // 4, (ch + 1) * n // 4)
                    nc.gpsimd.dma_start(kT[:, cs], kT_h[b][:, cs])
                    nc.gpsimd.dma_start(qT[:, cs], qT_h[b][:, cs])
                vp = vpp.tile([128, Kt * VP], BF16, tag="vp")
                nc.gpsimd.dma_start(vp, vp_h[b])

                o65 = o65p.tile([VW, n], BF16, tag="o65")
                ats = {}
                avs = {}

                def issue_qk(s, qT=qT, kT=kT, ats=ats):
                    qb, kt = divmod(s, Kt)
                    st = pst.tile([128, w], F32, tag="st")
                    for h in range(w // 512):
                        nc.tensor.matmul(
                            st[:, 512 * h : 512 * (h + 1)],
                            kT[:, 128 * kt : 128 * (kt + 1)],
                            qT[:, w * qb + 512 * h : w * qb + 512 * (h + 1)],
                            start=True,
                            stop=True,
                        )
                    araw = stage.tile([128, w], BF16, tag="araw")
                    nc.scalar.activation(araw, st, AF.Exp, scale=SCALE)
                    at = atp.tile([128, w], BF16, tag="at")
                    c0 = (n - 128) - 128 * kt + w * qb
                    nc.vector.tensor_mul(at, araw, expP[:, c0 : c0 + w])
                    ats[s] = at

                def issue_av(s, b=b, vp=vp, o65=o65, ats=ats, avs=avs):
                    qb, kt = divmod(s, Kt)
                    if kt == 0:
                        avs[qb] = pav.tile([128, w], F32, tag="avpj", name="av")
                    av = avs[qb]
                    at = ats.pop(s)
                    for h in range(w // 512):
                        nc.tensor.matmul(
                            av[:, 512 * h : 512 * (h + 1)],
                            vp[:, VP * kt : VP * (kt + 1)],
                            at[:, 512 * h : 512 * (h + 1)],
                            start=(kt == 0),
                            stop=(kt == Kt - 1),
                        )
                    if kt == Kt - 1:
                        nc.vector.tensor_copy(
                            o65[:, w * qb : w * (qb + 1)], av[0:VW, :]
                        )
                        del avs[qb]
                        # denominator bounce via casting DMA (bf16 -> f32;
                        # only gpsimd may cast)
                        nc.gpsimd.dma_start(
                            den_scr[b : b + 1, w * qb : w * (qb + 1)],
                            o65[HEAD_DIM : HEAD_DIM + 1, w * qb : w * (qb + 1)],
                        )
                        denT = recp.tile([128, qb_t], F32, tag="denT")
                        bsrc = bass.AP(
                            tensor=den_scr,
                            offset=b * n + w * qb,
                            ap=[[1, 128], [128, qb_t]],
                        )
                        nc.gpsimd.dma_start(denT, bsrc)
                        rec = recp.tile([128, qb_t], F32, tag="rec")
                        nc.vector.reciprocal(rec, denT)
                        for rj in range(qb_t):
                            proj_jobs.append(
                                make_pj(b, qb_t * qb + rj, o65, rec, rj)
                            )

                for s in range(nsteps + LAG):
                    if s < nsteps:
                        issue_qk(s)
                    if s >= LAG:
                        issue_av(s - LAG)
                    # drain one projection every 4 steps: the extra PE matmul
                    # per 4 steps stays under the ACT exp pace.  The last
                    # query block's jobs intentionally spill into the next
                    # batch's loop — draining them here would stall the PE on
                    # the reciprocal latency and MID-rethrottle the clock gate
                    if proj_jobs and s % 4 == 1:
                        proj_jobs.pop(0)()

            while proj_jobs:
                proj_jobs.pop(0)()

    nc.compile()
    return nc


def make_in_maps(q, k, v, rel_bias_table, W_out, n):
    """Shard + pre-layout full inputs per core (core c <-> head c)."""
    Kt = n // 128
    in_maps = []
    for c in range(N_CORES):
        sl = slice(HEAD_DIM * c, HEAD_DIM * (c + 1))
        qT = np.ascontiguousarray(np.transpose(q[:, :, sl], (0, 2, 1))).astype(
            ml_dtypes.bfloat16
        )
        kT = np.ascontiguousarray(np.transpose(k[:, :, sl], (0, 2, 1))).astype(
            ml_dtypes.bfloat16
        )
        vr = v[:, :, sl].reshape(2, Kt, 128, HEAD_DIM)
        vp = np.zeros((2, 128, Kt, 128), dtype=ml_dtypes.bfloat16)
        vp[:, :, :, :HEAD_DIM] = np.transpose(vr, (0, 2, 1, 3)).astype(
            ml_dtypes.bfloat16
        )
        vp[:, :, :, HEAD_DIM] = 1.0
        in_maps.append(
            {
                "qT_h": qT,
                "kT_h": kT,
                "vp_h": np.ascontiguousarray(vp.reshape(2, 128, Kt * 128)),
                "expP_h": _expP_np(rel_bias_table[:, c].astype(np.float64), n),
                "W_h": np.ascontiguousarray(W_out[sl, :]).astype(
                    ml_dtypes.bfloat16
                ),
            }
        )
    return in_maps


_NC_CACHE = {}


def _get_nc(n, w):
    key = (n, w)
    if key not in _NC_CACHE:
        _NC_CACHE[key] = build_nc(n=n, w=w)
    return _NC_CACHE[key]


def kernel(q, k, v, rel_bias_table, W_out, b_out):
    from concourse.bass_utils import run_bass_kernel_spmd

    q = np.asarray(q, dtype=np.float32)
    k = np.asarray(k, dtype=np.float32)
    v = np.asarray(v, dtype=np.float32)
    rel_bias_table = np.asarray(rel_bias_table, dtype=np.float32)
    W_out = np.asarray(W_out, dtype=np.float32)
    b_out = np.asarray(b_out, dtype=np.float32)

    n = q.shape[1]
    w = min(1024, n)
    nc = _get_nc(n, w)
    in_maps = make_in_maps(q, k, v, rel_bias_table, W_out, n)
    res = run_bass_kernel_spmd(nc, in_maps, core_ids=list(range(N_CORES)))
    acc = np.zeros((2, n, OUT_F), dtype=np.float64)
    for r in res.results:
        acc += r["out_partial"].astype(np.float64)
    return (acc + b_out.astype(np.float64)).astype(np.float32)
